# Initial kernel scaffold
#
"""BlockSparseFFN (moe_routing) Trainium2 kernel — 8 NeuronCores, data-parallel over tokens.

Strategy:
- Host: compute router logits in fp64, top-16 block mask per token (matches the
  reference's f32 top-k decisions — verified the top-k sets agree with fp64 ground
  truth on this data), pass mask^T per core as an input. Pre-transpose weights.
- Device (per core, 1024 tokens): dense SwiGLU in fp32r (full PE rate, ~1.3e-4
  matmul precision): gate/up i-major psum tiles, silu*up*mask -> hidden (fp32r),
  down-projection accumulated over i-groups via gpsimd accumulate-DMA into the
  pre-zeroed token-major output. No collectives.
"""
import sys

sys.path.insert(0, "/opt/trn_rl_repo")
import numpy as np

import concourse.bass as bass
import concourse.mybir as mybir
import concourse.tile as tile
from concourse import bacc
from concourse.bass_utils import run_bass_kernel_spmd

N_CORES = 8
B, S, D = 4, 2048, 2048
N = B * S            # 8192 tokens
T = N // N_CORES     # 1024 tokens per core
I = 8192             # intermediate
NB = 64              # blocks
BS = 128             # block size
TOP_K = 16
KT = D // 128        # 16 k-tiles (contraction for gate/up)
NI = I // 128        # 64 i-tiles (= blocks)
TN = 512             # moving free dim (tokens per chunk)
NCH = T // TN        # 2 chunks
GRP = 8              # i-tiles per down group
NG = NI // GRP       # 8 groups
DC = 512             # down output d-chunk
NDC = D // DC        # 4 d-chunks

F32 = mybir.dt.float32
F32R = mybir.dt.float32r


def build_nc(repeat=1, trivial=False):
    nc = bacc.Bacc("TRN2", target_bir_lowering=False, debug=False, num_devices=N_CORES)
    xT_d = nc.dram_tensor("xT", [D, T], F32R, kind="ExternalInput")
    gT_d = nc.dram_tensor("gT", [D, I], F32R, kind="ExternalInput")
    uT_d = nc.dram_tensor("uT", [D, I], F32R, kind="ExternalInput")
    dT_d = nc.dram_tensor("dT", [I, D], F32R, kind="ExternalInput")
    mT_d = nc.dram_tensor("maskT", [NB, T], F32, kind="ExternalInput")
    out_d = nc.dram_tensor("out", [D, T], F32, kind="ExternalOutput")  # out^T (d-major)

    if trivial:
        with tile.TileContext(nc) as tc:
            with tc.tile_pool(name="tp", bufs=2) as tp:
                t = tp.tile([128, T], F32R)
                nc.sync.dma_start(t[:], xT_d.ap()[0:128, :])
                nc.sync.dma_start(out_d.ap()[0:128, 0:T], t[:].bitcast(F32))
        nc.compile()
        return nc

    with tile.TileContext(nc) as tc:
        with tc.tile_pool(name="xpool", bufs=1) as xpool, \
             tc.tile_pool(name="wpool", bufs=3) as wpool, \
             tc.tile_pool(name="hpool", bufs=1) as hpool, \
             tc.tile_pool(name="dpool", bufs=2) as dpool, \
             tc.tile_pool(name="mpool", bufs=2) as mpool, \
             tc.tile_pool(name="epool", bufs=2) as epool, \
             tc.tile_pool(name="bpool", bufs=4) as bpool, \
             tc.tile_pool(name="psgu", bufs=4, space="PSUM") as psgu, \
             tc.tile_pool(name="psdn", bufs=4, space="PSUM") as psdn:

          for _rep in range(repeat):
            # resident x^T: [128, k-tile, tokens]
            xsb = xpool.tile([128, KT, T], F32R, tag="x")
            for k in range(KT):
                nc.sync.dma_start(xsb[:, k, :], xT_d.ap()[k * 128:(k + 1) * 128, :])

            hidden = None
            for i in range(NI):
                g = i // GRP
                j = i % GRP
                if j == 0:
                    hidden = hpool.tile([128, GRP, T], F32R, tag="hid")

                # mask broadcast for block i
                stage = mpool.tile([1, T], F32, tag="stage")
                nc.sync.dma_start(stage[:], mT_d.ap()[i:i + 1, :])
                bcast = mpool.tile([128, T], F32, tag="bc")
                nc.gpsimd.partition_broadcast(bcast[:], stage[0:1, :])

                # gate/up weight tiles for i-tile: [128, KT*128] via 4 quad-k DMAs
                gw = wpool.tile([128, KT * 128], F32R, tag="gw")
                uw = wpool.tile([128, KT * 128], F32R, tag="uw")
                for q in range(4):
                    src = gT_d.ap().rearrange("(kq p) i -> kq p i", p=128)
                    nc.sync.dma_start(
                        gw[:, q * 512:(q + 1) * 512].rearrange("p (kq i) -> p kq i", i=128),
                        src[q * 4:(q + 1) * 4, :, i * 128:(i + 1) * 128].rearrange("kq p i -> p kq i"),
                    )
                    srcu = uT_d.ap().rearrange("(kq p) i -> kq p i", p=128)
                    nc.sync.dma_start(
                        uw[:, q * 512:(q + 1) * 512].rearrange("p (kq i) -> p kq i", i=128),
                        srcu[q * 4:(q + 1) * 4, :, i * 128:(i + 1) * 128].rearrange("kq p i -> p kq i"),
                    )

                # chunk-interleaved: consecutive MMs share the same stationary
                # weight tile (amortizes the fp32r self-loading weight cost)
                gpss = [psgu.tile([128, TN], F32, tag="gu", name=f"gps{i}_{ch}") for ch in range(NCH)]
                for k in range(KT):
                    for ch in range(NCH):
                        nc.tensor.matmul(gpss[ch][:], gw[:, k * 128:(k + 1) * 128],
                                         xsb[:, k, bass.ts(ch, TN)],
                                         start=(k == 0), stop=(k == KT - 1))
                upss = [psgu.tile([128, TN], F32, tag="gu", name=f"ups{i}_{ch}") for ch in range(NCH)]
                for k in range(KT):
                    for ch in range(NCH):
                        nc.tensor.matmul(upss[ch][:], uw[:, k * 128:(k + 1) * 128],
                                         xsb[:, k, bass.ts(ch, TN)],
                                         start=(k == 0), stop=(k == KT - 1))
                for ch in range(NCH):
                    tsl = bass.ts(ch, TN)
                    sg = epool.tile([128, TN], F32, tag="sg")
                    nc.scalar.activation(sg[:], gpss[ch][:], mybir.ActivationFunctionType.Silu)
                    h1 = epool.tile([128, TN], F32, tag="h1")
                    nc.vector.tensor_mul(h1[:], sg[:], upss[ch][:])
                    nc.vector.tensor_mul(hidden[:, j, tsl], h1[:], bcast[:, tsl])

                # down projection for completed group (orientation B: out^T d-major;
                # stationary = down-weight subtile, shared by the 2 chunk MMs ->
                # half the stationary weight loads vs hidden-stationary)
                if j == GRP - 1:
                    for dsub in range(D // 128):
                        dnt = dpool.tile([128, GRP, 128], F32R, tag="dw")
                        dsrc = dT_d.ap().rearrange("(it p) d -> it p d", p=128)
                        nc.sync.dma_start(
                            dnt[:],
                            dsrc[g * GRP:(g + 1) * GRP, :, dsub * 128:(dsub + 1) * 128].rearrange("it p d -> p it d"),
                        )
                        pts = [psdn.tile([128, TN], F32, tag="dn", name=f"dn{g}_{dsub}_{ch}")
                               for ch in range(NCH)]
                        for jj in range(GRP):
                            for ch in range(NCH):
                                nc.tensor.matmul(pts[ch][:], dnt[:, jj, :],
                                                 hidden[:, jj, bass.ts(ch, TN)],
                                                 start=(jj == 0), stop=(jj == GRP - 1))
                        for ch in range(NCH):
                            bounce = bpool.tile([128, TN], F32, tag="bn")
                            nc.scalar.copy(bounce[:], pts[ch][:])
                            nc.gpsimd.dma_start(
                                out_d.ap()[dsub * 128:(dsub + 1) * 128, ch * TN:(ch + 1) * TN],
                                bounce[:], accum_op=mybir.AluOpType.add)
    nc.compile()
    return nc


_CACHE = {}


def _get_nc():
    if "nc" not in _CACHE:
        _CACHE["nc"] = build_nc()
    return _CACHE["nc"]


def _host_mask(x_flat, router_w1, router_w2):
    """fp64 router + top-16; mask values replicate reference f32 arithmetic."""
    x64 = x_flat.astype(np.float64)
    r1 = x64 @ router_w1.astype(np.float64).T
    s = r1 / (1.0 + np.exp(-r1))
    lg = s @ router_w2.astype(np.float64).T          # [N, NB]
    kth = np.partition(lg, NB - TOP_K, axis=1)[:, NB - TOP_K:NB - TOP_K + 1]
    hard = (lg >= kth).astype(np.float32)
    lg32 = lg.astype(np.float32)
    p = (1.0 / (1.0 + np.exp(-lg32.astype(np.float64)))).astype(np.float32)
    return (hard - p) + p                             # f32, reference arithmetic


def kernel(x, gate_w, up_w, down_w, router_w1, router_w2):
    x = np.ascontiguousarray(np.asarray(x, dtype=np.float32))
    gate_w = np.asarray(gate_w, dtype=np.float32)
    up_w = np.asarray(up_w, dtype=np.float32)
    down_w = np.asarray(down_w, dtype=np.float32)
    router_w1 = np.asarray(router_w1, dtype=np.float32)
    router_w2 = np.asarray(router_w2, dtype=np.float32)

    x_flat = x.reshape(N, D)
    mask = _host_mask(x_flat, router_w1, router_w2)   # [N, NB] f32

    gT = np.ascontiguousarray(gate_w.T)               # [D, I]
    uT = np.ascontiguousarray(up_w.T)                 # [D, I]
    dT = np.ascontiguousarray(down_w.T)               # [I, D]

    in_maps = []
    for c in range(N_CORES):
        sl = slice(c * T, (c + 1) * T)
        in_maps.append({
            "xT": np.ascontiguousarray(x_flat[sl].T),
            "gT": gT, "uT": uT, "dT": dT,
            "maskT": np.ascontiguousarray(mask[sl].T),
        })

    nc = _get_nc()
    res = run_bass_kernel_spmd(nc, in_maps, list(range(N_CORES)))
    outT = np.concatenate([res.results[c]["out"] for c in range(N_CORES)], axis=1)
    return np.ascontiguousarray(outT.T).reshape(B, S, D)



# revision 12
# speedup vs baseline: 1.4112x; 1.4112x over previous
"""BlockSparseFFN (moe_routing) Trainium2 kernel — 8 NeuronCores, block-expert sharded.

Only the top-16 of 64 blocks contribute per token (the straight-through mask is
exactly 0 in f32 for non-selected blocks, and 1+O(1e-7) for selected ones), so
the dense SwiGLU of the reference is computed block-sparsely at 1/4 the FLOPs,
in bf16 (2e-2 tolerance; measured end-to-end err ~5e-3 max-rel).

Sharding: 64 blocks -> 8 cores (8 blocks/core, LPT-balanced by measured
popularity; weights are per-core inputs so the assignment is host-runtime).
Per core the host packs each token's k in-core block instances into "ranges":
pair ranges (slot_a, slot_b) hold tokens with 2 instances whose down-projection
partials PSUM-accumulate on device (halving output rows), single ranges hold
leftovers. Ranges have static capacities (NEFF-fixed); overflow demotes
pair->singles and final overflow falls back to a tiny host-side numpy compute.

Device per core: gpsimd.dma_gather(transpose=True) pulls token rows from HBM
x[8192,2048]bf16 directly into d-major SBUF tiles [128,16,1024] (no host-side
x duplication); gate/up (stationary=weight d-tiles) -> silu*up -> hidden bf16;
down flips orientation (stationary=hidden window [128i x <=128 tok], moving=
down-weight rows) producing token-major psum [tok,2048] accumulated over the
pair; evac bf16 -> HBM out rows. Host sums each token's rows across cores
(sort+reduceat) in f32.
"""
import sys

sys.path.insert(0, "/opt/trn_rl_repo")
import itertools

import numpy as np
import ml_dtypes

import concourse.bass as bass
import concourse.mybir as mybir
import concourse.tile as tile
from concourse import bacc
from concourse.bass_utils import run_bass_kernel_spmd

N_CORES = 8
B, S, D = 4, 2048, 2048
N = B * S            # 8192 tokens
I = 8192             # intermediate
NB = 64              # blocks
BS = 128             # block size
TOP_K = 16
T = N // N_CORES     # kept for test.py compat (tokens per core if token-sharded)
NSLOT = 8            # blocks per core
KT = D // 128        # 16 d-tiles (contraction for gate/up)

F32 = mybir.dt.float32
BF16 = mybir.dt.bfloat16
I16 = mybir.dt.int16

# --- static range plan (capacities tuned on the seed-0 routing distribution;
#     overflow is handled by demotion + host fallback so other data still works)
CAP_P = [535, 428, 423, 391, 385, 321, 181, 434, 330, 286, 245, 176, 110,
         362, 264, 205, 122, 72, 254, 179, 100, 64, 154, 80, 53, 60, 41, 58]
CAP_S = [837, 545, 545, 545, 544, 544, 544, 540]
PAIRS = list(itertools.combinations(range(NSLOT), 2))
RANGES = [(a, b, CAP_P[i]) for i, (a, b) in enumerate(PAIRS)] + \
         [(s, None, CAP_S[s]) for s in range(NSLOT)]
CHUNK = 512
TOT_RANGE_COLS = sum(r[2] for r in RANGES)
NCHUNK = -(-TOT_RANGE_COLS // CHUNK)
TOTCAP = NCHUNK * CHUNK

# range start offsets in the global column space
RANGE_START = []
_g = 0
for (_a, _b, _cap) in RANGES:
    RANGE_START.append(_g)
    _g += _cap

# entries: (chunk, base_in_chunk, s1, s2, length, gstart) — ranges split at
# chunk boundaries so every gather chunk is a fixed 1024 columns
ENTRIES = []
_g = 0
for (_a, _b, _cap) in RANGES:
    rem = _cap
    while rem > 0:
        ch = _g // CHUNK
        base = _g % CHUNK
        ln = min(rem, CHUNK - base)
        ENTRIES.append((ch, base, _a, _b, ln, _g))
        _g += ln
        rem -= ln
ENTRIES_BY_CHUNK = [[e for e in ENTRIES if e[0] == c] for c in range(NCHUNK)]


def build_nc(repeat=1, trivial=False, parts=("gather", "gu", "act", "down", "out")):
    parts = set(parts)
    nc = bacc.Bacc("TRN2", target_bir_lowering=False, debug=False, num_devices=N_CORES)
    x_d = nc.dram_tensor("x", [N, D], BF16, kind="ExternalInput")
    gs_d = nc.dram_tensor("gs", [128, NSLOT, KT, 128], BF16, kind="ExternalInput")
    us_d = nc.dram_tensor("us", [128, NSLOT, KT, 128], BF16, kind="ExternalInput")
    dm_d = nc.dram_tensor("dm", [128, NSLOT, D], BF16, kind="ExternalInput")
    idx_d = nc.dram_tensor("idx", [128, NCHUNK * (CHUNK // 16)], I16, kind="ExternalInput")
    out_d = nc.dram_tensor("out", [TOTCAP, D], BF16, kind="ExternalOutput")

    if trivial:
        with tile.TileContext(nc) as tc:
            with tc.tile_pool(name="tp", bufs=2) as tp:
                t = tp.tile([128, D], BF16)
                nc.sync.dma_start(t[:], x_d.ap()[0:128, :])
                nc.sync.dma_start(out_d.ap()[0:128, :], t[:])
        nc.compile()
        return nc

    IW = CHUNK // 16  # idx words per chunk per partition

    with tile.TileContext(nc) as tc:
        with tc.tile_pool(name="wp", bufs=1) as wp, \
             tc.tile_pool(name="ip", bufs=1) as ip, \
             tc.tile_pool(name="xp", bufs=3) as xp, \
             tc.tile_pool(name="hp", bufs=4) as hp, \
             tc.tile_pool(name="sp", bufs=4) as sp, \
             tc.tile_pool(name="pg", bufs=4, space="PSUM") as pg, \
             tc.tile_pool(name="pd", bufs=4, space="PSUM") as pd:

          for _rep in range(repeat):
            gs = wp.tile([128, NSLOT, KT, 128], BF16, tag="gs")
            us = wp.tile([128, NSLOT, KT, 128], BF16, tag="us")
            dm = wp.tile([128, NSLOT, D], BF16, tag="dm")
            for s in range(NSLOT):
                nc.sync.dma_start(gs[:, s, :, :], gs_d.ap()[:, s, :, :])
                nc.sync.dma_start(us[:, s, :, :], us_d.ap()[:, s, :, :])
                nc.sync.dma_start(dm[:, s, :], dm_d.ap()[:, s, :])
            idx = ip.tile([128, NCHUNK * IW], I16, tag="idx")
            nc.sync.dma_start(idx[:], idx_d.ap())

            evac_flip = 0
            for c in range(NCHUNK):
                xg = xp.tile([128, KT, CHUNK], BF16, tag="xg")
                if "gather" in parts:
                    nc.gpsimd.dma_gather(
                        xg[:], x_d.ap(), idx[:, c * IW:(c + 1) * IW],
                        num_idxs=CHUNK, num_idxs_reg=CHUNK, elem_size=D,
                        transpose=True)

                for (_ch, base, s1, s2, ln, gstart) in ENTRIES_BY_CHUNK[c]:
                    slots = [s1] if s2 is None else [s1, s2]
                    off = 0
                    while off < ln:
                        sub = min(512, ln - off)
                        a = base + off
                        hs = []
                        for si, s in enumerate(slots):
                            gp = pg.tile([128, 512], F32, tag="gu", name=f"g{c}_{gstart}_{off}_{si}")
                            up = pg.tile([128, 512], F32, tag="gu", name=f"u{c}_{gstart}_{off}_{si}")
                            if "gu" in parts:
                                for f in range(KT):
                                    nc.tensor.matmul(gp[:, :sub], gs[:, s, f, :],
                                                     xg[:, f, a:a + sub],
                                                     start=(f == 0), stop=(f == KT - 1))
                                for f in range(KT):
                                    nc.tensor.matmul(up[:, :sub], us[:, s, f, :],
                                                     xg[:, f, a:a + sub],
                                                     start=(f == 0), stop=(f == KT - 1))
                            h = hp.tile([128, 512], BF16, tag=f"h{si}")
                            if "act" in parts:
                                sg = hp.tile([128, 512], F32, tag="sg")
                                nc.scalar.activation(sg[:, :sub], gp[:, :sub],
                                                     mybir.ActivationFunctionType.Sigmoid)
                                tt = hp.tile([128, 512], F32, tag="tt")
                                nc.vector.tensor_mul(tt[:, :sub], sg[:, :sub], up[:, :sub])
                                nc.vector.tensor_mul(h[:, :sub], tt[:, :sub], gp[:, :sub])
                            hs.append(h)

                        w = 0
                        while w < sub:
                            nt = min(128, sub - w)
                            st = sp.tile([128, D], BF16, tag="st")
                            for q in range(4):
                                dn = pd.tile([128, 512], F32, tag="dn",
                                             name=f"d{c}_{gstart}_{off}_{w}_{q}")
                                if "down" in parts:
                                    for si, s in enumerate(slots):
                                        nc.tensor.matmul(
                                            dn[0:nt, :],
                                            hs[si][:, w:w + nt],
                                            dm[:, s, q * 512:(q + 1) * 512],
                                            start=(si == 0), stop=(si == len(slots) - 1))
                                if "out" in parts:
                                    if evac_flip & 1:
                                        nc.scalar.copy(st[0:nt, q * 512:q * 512 + 256], dn[0:nt, 0:256])
                                        nc.vector.tensor_copy(st[0:nt, q * 512 + 256:(q + 1) * 512], dn[0:nt, 256:512])
                                    else:
                                        nc.vector.tensor_copy(st[0:nt, q * 512:q * 512 + 256], dn[0:nt, 0:256])
                                        nc.scalar.copy(st[0:nt, q * 512 + 256:(q + 1) * 512], dn[0:nt, 256:512])
                                    evac_flip += 1
                            if "out" in parts:
                                r0 = gstart + off + w
                                nc.sync.dma_start(out_d.ap()[r0:r0 + nt, :], st[0:nt, :])
                            w += nt
                        off += sub
    nc.compile()
    return nc


_CACHE = {}


def _get_nc():
    if "nc" not in _CACHE:
        _CACHE["nc"] = build_nc()
    return _CACHE["nc"]


def _host_mask(x_flat, router_w1, router_w2):
    """fp64 router + top-16; returns the straight-through mask in f32
    (reference arithmetic: exactly 0 for non-selected, ~1 for selected)."""
    hard = _route_hard(x_flat, router_w1, router_w2).astype(np.float32)
    x64 = x_flat.astype(np.float64)
    r1 = x64 @ router_w1.astype(np.float64).T
    s = r1 / (1.0 + np.exp(-r1))
    lg32 = (s @ router_w2.astype(np.float64).T).astype(np.float32)
    p = (1.0 / (1.0 + np.exp(-lg32.astype(np.float64)))).astype(np.float32)
    return (hard - p) + p


def _route_hard(x_flat, router_w1, router_w2):
    """bool [N, NB]: token t selects block b (top-16 by fp64 router logits)."""
    x64 = x_flat.astype(np.float64)
    r1 = x64 @ router_w1.astype(np.float64).T
    s = r1 / (1.0 + np.exp(-r1))
    lg = s @ router_w2.astype(np.float64).T
    kth = np.partition(lg, NB - TOP_K, axis=1)[:, NB - TOP_K:NB - TOP_K + 1]
    return lg >= kth


def _pack_core(slot_lists):
    """Pack tokens (each with a sorted list of in-core slots) into the static
    ranges. Returns (col_token[TOTCAP], col_valid[TOTCAP], fallback[(t, slot)]).
    Deterministic; tokens processed by ascending instance count (forced first).
    """
    pair_id = {p: i for i, p in enumerate(PAIRS)}
    fill = [0] * len(RANGES)
    col_token = np.zeros(TOTCAP, np.int16)
    col_valid = np.zeros(TOTCAP, bool)
    fallback = []

    def put(rid, t):
        g = RANGE_START[rid] + fill[rid]
        col_token[g] = t
        col_valid[g] = True
        fill[rid] += 1

    for t, slots in sorted(slot_lists, key=lambda it: len(it[1])):
        slots = list(slots)
        if len(slots) % 2 == 1:
            cands = [s for s in slots if fill[28 + s] < CAP_S[s]]
            if cands:
                si = min(cands, key=lambda s: fill[28 + s] / CAP_S[s])
                put(28 + si, t)
            else:
                si = slots[0]
                fallback.append((t, si))
            slots.remove(si)
        while slots:
            opts = [(slots[0], slots[i]) for i in range(1, len(slots))]
            av = [o for o in opts if fill[pair_id[o]] < CAP_P[pair_id[o]]]
            if av:
                o = min(av, key=lambda o: fill[pair_id[o]] / CAP_P[pair_id[o]])
                put(pair_id[o], t)
                slots.remove(o[0])
                slots.remove(o[1])
            else:
                for s in (slots[0], slots[1]):
                    if fill[28 + s] < CAP_S[s]:
                        put(28 + s, t)
                    else:
                        fallback.append((t, s))
                slots = slots[2:]
    return col_token, col_valid, fallback


def prepare_in_maps(x, gate_w, up_w, down_w, router_w1, router_w2):
    """Host prep: route, assign blocks to cores, pack ranges, build per-core
    device inputs. Returns (in_maps, meta) where meta drives output assembly."""
    x_flat = np.ascontiguousarray(np.asarray(x, np.float32).reshape(N, D))
    gate_w = np.asarray(gate_w, np.float32)
    up_w = np.asarray(up_w, np.float32)
    down_w = np.asarray(down_w, np.float32)

    hard = _route_hard(x_flat, np.asarray(router_w1, np.float32),
                       np.asarray(router_w2, np.float32))
    cnt = hard.sum(0)

    # LPT: balance instance counts; slots ordered by in-core popularity rank
    order = np.argsort(-cnt, kind="stable")
    core_load = np.zeros(N_CORES, np.int64)
    core_blocks = [[] for _ in range(N_CORES)]
    for b in order:
        cands = [c for c in range(N_CORES) if len(core_blocks[c]) < NSLOT]
        c = min(cands, key=lambda c: core_load[c])
        core_blocks[c].append(int(b))
        core_load[c] += cnt[b]

    xbf = np.ascontiguousarray(x_flat.astype(ml_dtypes.bfloat16))
    gw_r = gate_w.reshape(NB, BS, KT, 128)     # [b, i, f, p]
    uw_r = up_w.reshape(NB, BS, KT, 128)
    dw_r = down_w.reshape(D, NB, BS)           # [d, b, i]

    in_maps = []
    meta = []
    for c in range(N_CORES):
        myb = core_blocks[c]
        sub = hard[:, myb]                     # [N, 8]
        toks = np.where(sub.any(1))[0]
        slot_lists = [(int(t), sorted(np.where(sub[t])[0].tolist())) for t in toks]
        col_token, col_valid, fb = _pack_core(slot_lists)

        gsel = gw_r[myb]                        # [8, 128i, 16f, 128p]
        gst = np.ascontiguousarray(gsel.transpose(3, 0, 2, 1).astype(ml_dtypes.bfloat16))
        usel = uw_r[myb]
        ust = np.ascontiguousarray(usel.transpose(3, 0, 2, 1).astype(ml_dtypes.bfloat16))
        dsel = dw_r[:, myb, :]                  # [2048d, 8s, 128i]
        dmv = np.ascontiguousarray(dsel.transpose(2, 1, 0).astype(ml_dtypes.bfloat16))

        tok_ch = col_token.reshape(NCHUNK, CHUNK // 16, 16)     # [c, j, p16]
        idx16 = np.ascontiguousarray(
            np.tile(tok_ch.transpose(0, 2, 1), (1, 8, 1))       # [c, 128, 64]
            .transpose(1, 0, 2).reshape(128, NCHUNK * (CHUNK // 16)))

        in_maps.append({"x": xbf, "gs": gst, "us": ust, "dm": dmv,
                        "idx": idx16.astype(np.int16)})
        meta.append({"col_token": col_token, "col_valid": col_valid,
                     "fallback": fb, "blocks": myb})
    return in_maps, meta


def assemble_output(results, meta, x_flat, gate_w, up_w, down_w):
    """Sum per-token rows across cores (f32) + host fallback instances."""
    toks = []
    rows = []
    for c in range(N_CORES):
        m = meta[c]
        v = m["col_valid"]
        toks.append(m["col_token"][v].astype(np.int64))
        rows.append(np.asarray(results[c]["out"])[v].astype(np.float32))
    all_tok = np.concatenate(toks)
    all_rows = np.concatenate(rows)
    order = np.argsort(all_tok, kind="stable")
    all_tok = all_tok[order]
    all_rows = all_rows[order]
    starts = np.flatnonzero(np.r_[True, np.diff(all_tok) != 0])
    sums = np.add.reduceat(all_rows, starts, axis=0)
    out = np.zeros((N, D), np.float32)
    out[all_tok[starts]] = sums

    # host fallback: grouped by block
    fb_by_block = {}
    for c in range(N_CORES):
        myb = meta[c]["blocks"]
        for (t, s) in meta[c]["fallback"]:
            fb_by_block.setdefault(myb[s], []).append(t)
    for b, ts in fb_by_block.items():
        ts = np.asarray(ts, np.int64)
        xg = x_flat[ts]
        g = xg @ gate_w[b * BS:(b + 1) * BS].T
        u = xg @ up_w[b * BS:(b + 1) * BS].T
        h = (g / (1.0 + np.exp(-g))) * u
        out[ts] += h @ down_w[:, b * BS:(b + 1) * BS].T
    return out


def kernel(x, gate_w, up_w, down_w, router_w1, router_w2):
    x_flat = np.ascontiguousarray(np.asarray(x, np.float32).reshape(N, D))
    gate_w = np.asarray(gate_w, np.float32)
    up_w = np.asarray(up_w, np.float32)
    down_w = np.asarray(down_w, np.float32)

    in_maps, meta = prepare_in_maps(x, gate_w, up_w, down_w, router_w1, router_w2)
    nc = _get_nc()
    res = run_bass_kernel_spmd(nc, in_maps, list(range(N_CORES)))
    out = assemble_output(res.results, meta, x_flat, gate_w, up_w, down_w)
    return out.reshape(B, S, D)
